# revision 1
# baseline (speedup 1.0000x reference)
"""Trainium2 Bass kernel for CantorGlobalAttention.

Math (per direction d, expert e, batch b, patch p):
  s_w   = Ksum[d, routes[e,w], b] * scale * beta[e,w]      (w = 0..2)
  v_w   = Vmean[d, routes[e,w], b]
  out_d = softmax_w(q * s_w) . v
Final output = mean over d, laid out [B, E*P].

Since the softmax is over only W=3 scalars that multiply the same q, it
collapses to an elementwise function of q with 6 per-row constants:
  sg  = sigmoid(q*(s0-s1)) = 0.5 + 0.5*tanh(q*(s0-s1)/2)
  A   = v1 + (v0-v1)*sg                       (exact 2-way softmax of {0,1})
  s2  = 1/(1 + e^{q*(s0-s2)} + e^{q*(s1-s2)}) (true 3-way weight of w=2)
  out_d = A + (v2 - A)*s2
Rewritten with tanh (so all ACT funcs live in one table set):
  H  = 0.5*(v0-v1);  Cp = (v2-v1) - H;  T = Cp - H*tanh(q*d01h)
  out_d = v2 - T + T*s2
Accumulated over d on the PE via identity matmuls into PSUM:
  OUT = 0.2 * (sum_d (z_d - T_d)) + 0.2*sum_d v2_d,   z = T*s2

All the per-row constants are produced on-device by tiny one-hot matmuls
(host-built gather matrices folded with scale/beta/0.5 factors) applied to
Ksum / Vsum columns (one col per direction).

Sharding: data-parallel over batch (dim 2), 8 cores x 8 batches.
"""

import numpy as np
from contextlib import ExitStack

import concourse.bass as bass
import concourse.bacc as bacc
import concourse.tile as tile
from concourse import mybir
from concourse import bass_utils

F32 = mybir.dt.float32
AF = mybir.ActivationFunctionType
OP = mybir.AluOpType

D, E, B, P = 5, 16, 64, 4096
W = 3
NCORES = 8
BC = B // NCORES          # 8 batches per core
R = E * BC                # 128 rows = partitions, r = e*BC + b
EXPERT_DIM = 128

CKV = 1024                # K/V reduce chunk (cols)
CQ = 1024                 # Q-phase chunk (cols)
MMF = 512                 # matmul max free dim

CLAMP = 1e37              # keep reciprocal_approx_fast input finite


def _build_bass():
    nc = bacc.Bacc("TRN2", debug=False, num_devices=NCORES)
    # chunk-major layouts: every tile transfer is one contiguous DRAM block
    q = nc.dram_tensor("q", [D, P // CQ, R, CQ], F32,
                       kind="ExternalInput").ap()
    k = nc.dram_tensor("k", [D, P // CKV, R, CKV], F32,
                       kind="ExternalInput").ap()
    v = nc.dram_tensor("v", [D, P // CKV, R, CKV], F32,
                       kind="ExternalInput").ap()
    # 6 gather matrices (pre-transposed for lhsT) + I and -I
    mats = nc.dram_tensor("mats", [8, R, R], F32, kind="ExternalInput").ap()
    out = nc.dram_tensor("out", [P // CQ, R, CQ], F32,
                         kind="ExternalOutput").ap()

    with ExitStack() as ctx:
        tc = ctx.enter_context(tile.TileContext(nc))
        _body(ctx, tc, q, k, v, mats, out)
    if not nc.is_finalized():
        nc.finalize()
    return nc


def _body(ctx, tc, q, k, v, mats, out):
    nc = tc.nc
    singles = ctx.enter_context(tc.tile_pool(name="singles", bufs=1))

    # Long-lived Q-phase pools are created BEFORE the short-lived K/V pool:
    # the stack allocator then never hands the K/V zone to Q-phase tiles,
    # which would serialize the Q pipeline behind the last V reduce
    # (released-zone dependency).
    qpool = ctx.enter_context(tc.tile_pool(name="qp", bufs=8))
    work = ctx.enter_context(tc.tile_pool(name="work", bufs=6))
    th_pool = ctx.enter_context(tc.tile_pool(name="thp", bufs=6))
    outp = ctx.enter_context(tc.tile_pool(name="outp", bufs=2))
    acc_pool = ctx.enter_context(tc.tile_pool(name="accp", bufs=3,
                                              space="PSUM"))

    # All input loads go through the SP trigger stream: its FIFO gives
    # strict priority ordering (K+Qpre interleaved per d -> V -> Q rest) and
    # the DGE spreads transfers over all 16 HW queues regardless of engine.
    def load(dst, src_ap):
        return nc.sync.dma_start(out=dst, in_=src_ap)

    # --- constants to SBUF ---
    mat_sb = []
    for i in range(8):
        m = singles.tile([R, R], F32, tag=f"mat{i}")
        load(m, mats[i, :, :])
        mat_sb.append(m)
    (m_d01h, m_d02, m_d12, m_hn, m_cp, m_v2, m_pos, m_neg) = mat_sb

    nkv = P // CKV
    # per-direction constants ([R,1] tiles, one set per d)
    d01h = [singles.tile([R, 1], F32, tag=f"d01h{d}", name=f"d01h{d}")
            for d in range(D)]
    d02 = [singles.tile([R, 1], F32, tag=f"d02{d}", name=f"d02{d}")
            for d in range(D)]
    d12 = [singles.tile([R, 1], F32, tag=f"d12{d}", name=f"d12{d}")
            for d in range(D)]
    hn = [singles.tile([R, 1], F32, tag=f"hn{d}", name=f"hn{d}")
            for d in range(D)]
    cp = [singles.tile([R, 1], F32, tag=f"cp{d}", name=f"cp{d}")
            for d in range(D)]
    v2 = [singles.tile([R, 1], F32, tag=f"v2{d}", name=f"v2{d}")
            for d in range(D)]
    c2s = singles.tile([R, 1], F32, tag="c2s")
    qpre = []

    with tc.tile_pool(name="kv", bufs=4) as kv_pool, \
         tc.tile_pool(name="part", bufs=4) as part_pool, \
         tc.tile_pool(name="prep", bufs=2, space="PSUM") as pre_psum:

        def reduce_d(name, src, d, sum_col, act_share):
            # reduce src[d] (nkv contiguous chunks) into sum_col [R,1]
            parts = part_pool.tile([R, nkv], F32, tag=f"{name}p")
            for c in range(nkv):
                t = kv_pool.tile([R, CKV], F32, tag=name)
                load(t, src[d, c, :, :])
                if act_share and c % 2 == 0:
                    nc.scalar.activation(out=t, in_=t, func=AF.Copy,
                                         accum_out=parts[:, c:c + 1])
                else:
                    nc.vector.tensor_reduce(out=parts[:, c:c + 1], in_=t,
                                            axis=mybir.AxisListType.X,
                                            op=OP.add)
            nc.vector.tensor_add(parts[:, 0:1], parts[:, 0:1], parts[:, 1:2])
            nc.vector.tensor_add(parts[:, 2:3], parts[:, 2:3], parts[:, 3:4])
            nc.vector.tensor_add(sum_col, parts[:, 0:1], parts[:, 2:3])

        def prelude_d(sum_col, pairs):
            for lhsT, dst in pairs:
                pt = pre_psum.tile([R, 1], F32, tag="pre")
                nc.tensor.matmul(pt, lhsT, sum_col, start=True, stop=True)
                nc.vector.tensor_copy(dst, pt)

        # K, V and the first Q chunk interleaved per direction: all the
        # data that gates direction d's full pipeline (including T, which
        # needs vsum[d]) arrives together and early; the remaining Q chunks
        # stream last, when the engines are the bottleneck, not the DMA
        for d in range(D):
            ks = singles.tile([R, 1], F32, tag=f"ks{d}")
            reduce_d("k", k, d, ks, act_share=False)
            vs = singles.tile([R, 1], F32, tag=f"vs{d}")
            reduce_d("v", v, d, vs, act_share=True)
            qt = qpool.tile([R, CQ], F32, tag="q")
            load(qt, q[d, 0, :, :])
            qpre.append(qt)
            prelude_d(ks, ((m_d01h, d01h[d]), (m_d02, d02[d]),
                           (m_d12, d12[d])))
            prelude_d(vs, ((m_hn, hn[d]), (m_cp, cp[d]), (m_v2, v2[d])))

    nc.vector.tensor_add(c2s, v2[0], v2[1])
    nc.vector.tensor_add(c2s, c2s, v2[2])
    nc.vector.tensor_add(c2s, c2s, v2[3])
    nc.vector.tensor_add(c2s, c2s, v2[4])
    nc.vector.tensor_scalar_mul(c2s, c2s, 1.0 / D)

    # --- Q phase ---
    nq = P // CQ
    for c in range(nq):
        acc = acc_pool.tile([R, CQ], F32, tag="acc")
        for d in range(D):
            if c == 0:
                qt = qpre[d]
            else:
                qt = qpool.tile([R, CQ], F32, tag="q")
                nc.sync.dma_start(out=qt, in_=q[d, c, :, :])
            th = th_pool.tile([R, CQ], F32, tag="th")
            nc.scalar.activation(out=th, in_=qt, func=AF.Tanh, scale=d01h[d])
            ea = work.tile([R, CQ], F32, tag="ea")
            nc.scalar.activation(out=ea, in_=qt, func=AF.Exp, scale=d02[d])
            eb = work.tile([R, CQ], F32, tag="eb")
            nc.scalar.activation(out=eb, in_=qt, func=AF.Exp, scale=d12[d])
            # s2 = 1/min(1 + ea + eb, CLAMP); reuse ea/eb storage in place
            nc.gpsimd.tensor_tensor(ea, ea, eb, OP.add)
            nc.vector.tensor_scalar(ea, ea, 1.0, CLAMP, OP.add, OP.min)
            nc.vector.reciprocal_approx_fast(out=eb, in_=ea)  # eb := s2
            # T = Cp - H*th ; alternate engine for load balance
            tt = work.tile([R, CQ], F32, tag="tt")
            if (c * D + d) % 2 == 0:
                nc.scalar.activation(out=tt, in_=th, func=AF.Identity,
                                     scale=hn[d], bias=cp[d])
            else:
                nc.vector.tensor_scalar(tt, th, hn[d], cp[d],
                                        OP.mult, OP.add)
            # w = (s2-1)*T = z - T in one stt; single PE stream
            nc.vector.scalar_tensor_tensor(out=th, in0=eb, scalar=1.0,
                                           in1=tt, op0=OP.subtract,
                                           op1=OP.mult)  # th := w
            for pc in range(CQ // MMF):
                sl = slice(pc * MMF, (pc + 1) * MMF)
                nc.tensor.matmul(acc[:, sl], m_pos, th[:, sl],
                                 start=(d == 0), stop=(d == D - 1))
        osb = outp.tile([R, CQ], F32, tag="osb")
        nc.scalar.activation(out=osb, in_=acc, func=AF.Identity,
                             scale=1.0 / D, bias=c2s[:, 0:1])
        nc.scalar.dma_start(out=out[c, :, :], in_=osb)


def _host_constants(betas, temperature, routes):
    """Build the 6 gather matrices (+-I) from the tiny replicated inputs."""
    betas = np.asarray(betas, dtype=np.float32)
    routes = np.asarray(routes).astype(np.int64)
    temp = np.abs(np.asarray(temperature, dtype=np.float32).reshape(-1)[0])
    scale = np.float32(1.0) / (np.sqrt(np.float32(EXPERT_DIM)) * temp)

    self_idx = np.arange(E)
    gate = np.where(
        routes == self_idx[:, None], np.float32(1.0),
        (np.float32(1.0) / (np.float32(1.0) +
                            np.exp(-betas[self_idx[:, None], routes]))),
    ).astype(np.float32)  # [E, W]

    A = np.zeros((W, R, R), dtype=np.float32)   # s_w gather (scale*beta folded)
    G = np.zeros((W, R, R), dtype=np.float32)   # v_w gather (1/P folded)
    rows = np.arange(R)
    e_of_r = rows // BC
    b_of_r = rows % BC
    for w in range(W):
        cols = routes[e_of_r, w] * BC + b_of_r
        A[w, rows, cols] += scale * gate[e_of_r, w]
        G[w, rows, cols] += np.float32(1.0 / P)

    m_d01h = 0.5 * (A[0] - A[1])
    m_d02 = A[0] - A[2]
    m_d12 = A[1] - A[2]
    m_h = 0.5 * (G[0] - G[1])
    m_hn = -m_h
    m_cp = (G[2] - G[1]) - m_h
    m_v2 = G[2]
    eye = np.eye(R, dtype=np.float32)
    mats = np.stack([m_d01h.T, m_d02.T, m_d12.T, m_hn.T, m_cp.T, m_v2.T,
                     eye, -eye]).astype(np.float32)
    return np.ascontiguousarray(mats)


_CACHE = {}


def kernel(Q, K, V, betas, temperature, routes, num_patches):
    Q = np.asarray(Q, dtype=np.float32)
    K = np.asarray(K, dtype=np.float32)
    V = np.asarray(V, dtype=np.float32)
    mats = _host_constants(betas, temperature, routes)

    if "nc" not in _CACHE:
        _CACHE["nc"] = _build_bass()
    nc = _CACHE["nc"]

    def shard(X, C):
        # [D,E,B,P] batch-slice -> chunk-major [D, P//C, R, C], contiguous
        outs = []
        for i in range(NCORES):
            sl = X[:, :, i * BC:(i + 1) * BC, :].reshape(D, R, P // C, C)
            outs.append(np.ascontiguousarray(sl.transpose(0, 2, 1, 3)))
        return outs

    qs, ks, vs = shard(Q, CQ), shard(K, CKV), shard(V, CKV)
    in_maps = [{"q": qs[i], "k": ks[i], "v": vs[i], "mats": mats}
               for i in range(NCORES)]

    res = bass_utils.run_bass_kernel_spmd(nc, in_maps,
                                          core_ids=list(range(NCORES)))
    _CACHE["last"] = res
    # device out: [P//CQ, R, CQ] with r = e*BC + b -> [BC, E*P]
    full = np.empty((B, E * P), dtype=np.float32)
    nq = P // CQ
    for i in range(NCORES):
        o = res.results[i]["out"].reshape(nq, E, BC, CQ)
        full[i * BC:(i + 1) * BC] = (
            o.transpose(2, 1, 0, 3).reshape(BC, E * P))
    return full



# revision 6
# speedup vs baseline: 1.2177x; 1.2177x over previous
"""Trainium2 Bass kernel for CantorGlobalAttention (v2: bf16 + PE reduce).

Math (per direction d, row r=(e,b), patch p):
  th  = tanh(q * d01h)                      (2-way blend, overflow-free)
  s2  = 1/min(1 + e^{q*d02} + e^{q*d12}, C) (3rd-neighbor weight)
  T   = Cp - H*th       (= v2 - A, A = 2-way softmax blend of v0,v1)
  out_d = v2 - T*(1-s2)
  OUT = mean_d out_d = c2s - sum_d (T_d/5)*(1-s2_d)

v2 changes vs v1:
  - Q/K/V uploaded as bf16 (half the HBM traffic; tolerance is 2e-2).
  - K/V row-sums computed on the Tensor engine: K^T/V^T uploaded
    transposed+chunk-packed, column-summed against a ones vector with
    PSUM accumulation; row->column via tiny k=1 matmuls.
  - Q-phase element ops in bf16 (DVE 2x/4x modes), spread over
    Scalar (3 ACTs), GPSIMD (add + T), DVE (clamp, recip, 1-s2, mult).
  - Per-direction mean accumulated on PE via -I identity matmuls.
  - Output written bf16, cast to f32 on host.

Sharding: data-parallel over batch (dim 2), 8 cores x 8 batches.
"""

import numpy as np
import ml_dtypes
from contextlib import ExitStack

import concourse.bass as bass
import concourse.bacc as bacc
import concourse.tile as tile
from concourse import mybir
from concourse import bass_utils

F32 = mybir.dt.float32
BF16 = mybir.dt.bfloat16
AF = mybir.ActivationFunctionType
OP = mybir.AluOpType

D, E, B, P = 5, 16, 64, 4096
W = 3
NCORES = 8
BC = B // NCORES          # 8 batches per core
R = E * BC                # 128 rows = partitions, r = e*BC + b
EXPERT_DIM = 128

PH = 2048                 # phase width (cols per phase), P = 2*PH
NKV = 16                  # kv chunk-pairs per direction (32 blocks of 128)
MMF = 512                 # matmul max free dim (one PSUM bank)
CLAMP = 1e37


def _build_bass():
    nc = bacc.Bacc("TRN2", debug=False, num_devices=NCORES)
    q = nc.dram_tensor("q", [D, 2, R, PH], BF16, kind="ExternalInput").ap()
    # kvt[d, i, p, j, 0:128]=K^T block (2i+j), [128:256]=V^T block (2i+j)
    kvt = nc.dram_tensor("kvt", [D, NKV, 128, 2, 256], BF16,
                         kind="ExternalInput").ap()
    # 8 matrices: d01hT, d02T, d12T, hn5T, cp5T, v25T, -I, ones
    mats = nc.dram_tensor("mats", [8, R, R], BF16, kind="ExternalInput").ap()
    out = nc.dram_tensor("out", [2, R, PH], BF16, kind="ExternalOutput").ap()

    with ExitStack() as ctx:
        tc = ctx.enter_context(tile.TileContext(nc))
        _body(ctx, tc, q, kvt, mats, out)
    if not nc.is_finalized():
        nc.finalize()
    return nc


def _body(ctx, tc, q, kvt, mats, out):
    nc = tc.nc
    singles = ctx.enter_context(tc.tile_pool(name="singles", bufs=1))

    # pools (SBUF)
    qpool = ctx.enter_context(tc.tile_pool(name="qp", bufs=3))
    kv_pool = ctx.enter_context(tc.tile_pool(name="kv", bufs=16))
    wpool = ctx.enter_context(tc.tile_pool(name="wp", bufs=2))
    fpool = ctx.enter_context(tc.tile_pool(name="fp", bufs=2))
    opool = ctx.enter_context(tc.tile_pool(name="op", bufs=2))
    # pools (PSUM): acc 4 banks, red 1, tp 1, pp 1, c2s 1
    accp = ctx.enter_context(tc.tile_pool(name="accp", bufs=1, space="PSUM"))
    redp = ctx.enter_context(tc.tile_pool(name="redp", bufs=2, space="PSUM"))
    tpp = ctx.enter_context(tc.tile_pool(name="tpp", bufs=1, space="PSUM"))
    ppp = ctx.enter_context(tc.tile_pool(name="ppp", bufs=1, space="PSUM"))

    # --- constants ---
    mat_sb = []
    for i in range(8):
        m = singles.tile([R, R], BF16, tag=f"mat{i}")
        nc.sync.dma_start(out=m, in_=mats[i, :, :])
        mat_sb.append(m)
    (m_d01h, m_d02, m_d12, m_hn5, m_cp5, m_v25, m_negI, m_ones) = mat_sb
    ones_col = m_ones[:, 0:1]                 # bf16 [128,1] of 1.0
    one_f32 = singles.tile([1, 1], F32, tag="one32")
    nc.vector.memset(one_f32, 1.0)

    consts = [singles.tile([R, 6], F32, tag=f"c{d}", name=f"c{d}")
              for d in range(D)]
    kvc = [singles.tile([R, 2], BF16, tag=f"kvc{d}", name=f"kvc{d}")
           for d in range(D)]
    c2s = singles.tile([R, 1], F32, tag="c2s")

    def emit_reduce(d):
        red = redp.tile([1, 512], F32, tag="red")
        for i in range(NKV):
            t = kv_pool.tile([128, 2, 256], BF16, tag="kvt")
            nc.sync.dma_start(out=t, in_=kvt[d, i, :, :, :])
            nc.tensor.matmul(red, ones_col, t,
                             start=(i == 0), stop=(i == NKV - 1))
        # psum row -> sbuf row (GPSIMD), then k=1 matmuls fold the
        # even/odd block halves while transposing row->column
        row = wpool.tile([1, 512], F32, tag="row")
        nc.vector.tensor_copy(row, red)
        tp = tpp.tile([R, 2], F32, tag="tp")
        nc.tensor.matmul(tp[:, 0:1], row[0:1, 0:128], one_f32,
                         start=True, stop=False)
        nc.tensor.matmul(tp[:, 0:1], row[0:1, 256:384], one_f32,
                         start=False, stop=True)
        nc.tensor.matmul(tp[:, 1:2], row[0:1, 128:256], one_f32,
                         start=True, stop=False)
        nc.tensor.matmul(tp[:, 1:2], row[0:1, 384:512], one_f32,
                         start=False, stop=True)
        nc.vector.tensor_copy(kvc[d], tp)     # f32 psum -> bf16 cols
        kc, vc = kvc[d][:, 0:1], kvc[d][:, 1:2]
        # 6 prelude matvecs into one psum tile, one copy out
        pp = ppp.tile([R, 6], F32, tag="pp")
        for j, (mm, col) in enumerate(((m_d01h, kc), (m_d02, kc),
                                       (m_d12, kc), (m_hn5, vc),
                                       (m_cp5, vc))):
            nc.tensor.matmul(pp[:, j:j + 1], mm, col, start=True, stop=True)
        nc.tensor.matmul(pp[:, 5:6], m_v25, vc, start=True, stop=True)
        nc.vector.tensor_copy(consts[d], pp)

    def emit_qphase(d, ph, acc):
        cd = consts[d]
        d01h, d02, d12 = cd[:, 0:1], cd[:, 1:2], cd[:, 2:3]
        hn5, cp5 = cd[:, 3:4], cd[:, 4:5]
        qt = qpool.tile([R, PH], BF16, tag="q")
        nc.sync.dma_start(out=qt, in_=q[d, ph, :, :])
        th = wpool.tile([R, PH], BF16, tag="th")
        nc.scalar.activation(out=th, in_=qt, func=AF.Tanh, scale=d01h)
        ea = wpool.tile([R, PH], BF16, tag="ea")
        nc.scalar.activation(out=ea, in_=qt, func=AF.Exp, scale=d02)
        eb = wpool.tile([R, PH], BF16, tag="eb")
        nc.scalar.activation(out=eb, in_=qt, func=AF.Exp, scale=d12)
        s = wpool.tile([R, PH], BF16, tag="s")
        nc.gpsimd.tensor_tensor(s, ea, eb, OP.add)
        t5 = wpool.tile([R, PH], BF16, tag="t5")
        nc.gpsimd.tensor_scalar(out=t5, in0=th, scalar1=hn5, scalar2=cp5,
                                op0=OP.mult, op1=OP.add)
        z = fpool.tile([R, PH], F32, tag="z")
        nc.vector.tensor_scalar(out=z, in0=s, scalar1=1.0, scalar2=CLAMP,
                                op0=OP.add, op1=OP.min)
        r = fpool.tile([R, PH], F32, tag="r")
        nc.vector.reciprocal_approx_fast(out=r, in_=z)
        u = wpool.tile([R, PH], BF16, tag="u")
        nc.vector.tensor_scalar(out=u, in0=r, scalar1=-1.0, scalar2=1.0,
                                op0=OP.mult, op1=OP.add)
        y4 = wpool.tile([R, PH], BF16, tag="y4")
        nc.vector.tensor_tensor(y4, t5, u, OP.mult)
        for pc in range(PH // MMF):
            sl = slice(pc * MMF, (pc + 1) * MMF)
            nc.tensor.matmul(acc[:, sl], m_negI, y4[:, sl],
                             start=(d == 0), stop=(d == D - 1))

    def emit_phase_out(ph, acc):
        osb = opool.tile([R, PH], BF16, tag="osb")
        nc.vector.tensor_scalar(out=osb, in0=acc, scalar1=1.0,
                                scalar2=c2s[:, 0:1], op0=OP.mult, op1=OP.add)
        nc.scalar.dma_start(out=out[ph, :, :], in_=osb)

    # --- schedule: reduces staggered two directions ahead of phase A ---
    emit_reduce(0)
    emit_reduce(1)
    accA = accp.tile([R, PH], F32, tag="acc", name="accA")
    for d in range(D):
        emit_qphase(d, 0, accA)
        if d + 2 < D:
            emit_reduce(d + 2)
    nc.vector.tensor_add(c2s, consts[0][:, 5:6], consts[1][:, 5:6])
    nc.vector.tensor_add(c2s, c2s, consts[2][:, 5:6])
    nc.vector.tensor_add(c2s, c2s, consts[3][:, 5:6])
    nc.vector.tensor_add(c2s, c2s, consts[4][:, 5:6])
    emit_phase_out(0, accA)
    accB = accp.tile([R, PH], F32, tag="acc", name="accB")
    for d in range(D):
        emit_qphase(d, 1, accB)
    emit_phase_out(1, accB)


def _host_constants(betas, temperature, routes):
    betas = np.asarray(betas, dtype=np.float32)
    routes = np.asarray(routes).astype(np.int64)
    temp = np.abs(np.asarray(temperature, dtype=np.float32).reshape(-1)[0])
    scale = np.float32(1.0) / (np.sqrt(np.float32(EXPERT_DIM)) * temp)

    self_idx = np.arange(E)
    gate = np.where(
        routes == self_idx[:, None], np.float32(1.0),
        (np.float32(1.0) / (np.float32(1.0) +
                            np.exp(-betas[self_idx[:, None], routes]))),
    ).astype(np.float32)  # [E, W]

    A = np.zeros((W, R, R), dtype=np.float32)   # s_w gather (scale*beta)
    G = np.zeros((W, R, R), dtype=np.float32)   # v_w gather (1/P folded)
    rows = np.arange(R)
    e_of_r = rows // BC
    b_of_r = rows % BC
    for w in range(W):
        cols = routes[e_of_r, w] * BC + b_of_r
        A[w, rows, cols] += scale * gate[e_of_r, w]
        G[w, rows, cols] += np.float32(1.0 / P)

    m_d01h = 0.5 * (A[0] - A[1])
    m_d02 = A[0] - A[2]
    m_d12 = A[1] - A[2]
    m_h = 0.5 * (G[0] - G[1])
    m_hn5 = -m_h / D
    m_cp5 = ((G[2] - G[1]) - m_h) / D
    m_v25 = G[2] / D
    negI = -np.eye(R, dtype=np.float32)
    ones = np.ones((R, R), dtype=np.float32)
    mats = np.stack([m_d01h.T, m_d02.T, m_d12.T, m_hn5.T, m_cp5.T, m_v25.T,
                     negI, ones])
    return np.ascontiguousarray(mats).astype(ml_dtypes.bfloat16)


_CACHE = {}


def kernel(Q, K, V, betas, temperature, routes, num_patches):
    Q = np.asarray(Q, dtype=np.float32)
    K = np.asarray(K, dtype=np.float32)
    V = np.asarray(V, dtype=np.float32)
    mats = _host_constants(betas, temperature, routes)

    if "nc" not in _CACHE:
        _CACHE["nc"] = _build_bass()
    nc = _CACHE["nc"]

    in_maps = []
    for i in range(NCORES):
        sl = slice(i * BC, (i + 1) * BC)
        Qc = Q[:, :, sl, :].reshape(D, R, P)
        Kc = K[:, :, sl, :].reshape(D, R, P)
        Vc = V[:, :, sl, :].reshape(D, R, P)
        qh = np.ascontiguousarray(
            Qc.reshape(D, R, 2, PH).transpose(0, 2, 1, 3)
        ).astype(ml_dtypes.bfloat16)
        # K^T/V^T blocks: [D, 32, 128, 128] -> packed [D, 16, 128, 2, 256]
        Kt = Kc.transpose(0, 2, 1).reshape(D, 32, 128, 128)
        Vt = Vc.transpose(0, 2, 1).reshape(D, 32, 128, 128)
        kvb = np.concatenate([Kt, Vt], axis=-1)          # [D, 32, 128, 256]
        kvh = np.ascontiguousarray(
            kvb.reshape(D, NKV, 2, 128, 256).transpose(0, 1, 3, 2, 4)
        ).astype(ml_dtypes.bfloat16)
        in_maps.append({"q": qh, "kvt": kvh, "mats": mats})

    res = bass_utils.run_bass_kernel_spmd(nc, in_maps,
                                          core_ids=list(range(NCORES)))
    _CACHE["last"] = res
    full = np.empty((B, E * P), dtype=np.float32)
    for i in range(NCORES):
        o = res.results[i]["out"].astype(np.float32)     # [2, R, PH]
        full[i * BC:(i + 1) * BC] = (
            o.reshape(2, E, BC, PH).transpose(2, 1, 0, 3).reshape(BC, E * P))
    return full
